# revision 38
# baseline (speedup 1.0000x reference)
import sys, os
sys.path.insert(0, '/opt/trn_rl_repo')
import numpy as np
import ml_dtypes
import concourse.bass as bass
import concourse.bacc as bacc
import concourse.mybir as mybir
import concourse.tile as tile
from concourse.tile import add_dep_helper
from concourse.bass_utils import run_bass_kernel_spmd

BF = mybir.dt.bfloat16
F32 = mybir.dt.float32
F8 = mybir.dt.float8e4
AF = mybir.ActivationFunctionType
ALU = mybir.AluOpType
AX = mybir.AxisListType

N_CORES = 8
B = 512
BL = B // N_CORES      # 64 batch rows per core
T = 365
D = 256
FA = 128               # attention MLP hidden
NF = 16                # forecast steps
NR = 256               # decoder hidden

# max-phase split: DVE handles t[0:TDVE) in 4 chunks, Pool t[TDVE:365)
TDVE = 264
DCH = TDVE // 4        # 66


def _bf(x):
    return np.ascontiguousarray(x).astype(ml_dtypes.bfloat16)


def _f32(x):
    return np.ascontiguousarray(x).astype(np.float32)


def _ktile(w, k_total, pad_to):
    """[K, M] -> [ntiles, 128, M] zero-padded on K."""
    K, M = w.shape
    assert K == k_total
    nt = (pad_to + 127) // 128
    out = np.zeros((nt, 128, M), w.dtype)
    for i in range(nt):
        lo = i * 128
        hi = min(K, lo + 128)
        if lo < K:
            out[i, :hi - lo] = w[lo:hi]
    return out


def build_bass():
    nc = bacc.Bacc("TRN2", target_bir_lowering=False, debug=False,
                   num_devices=N_CORES)

    def inp(name, shape, dt=BF):
        return nc.dram_tensor(name, shape, dt, kind="ExternalInput")

    # per-core sharded tensors
    xdt = inp("xdt", [2, 128, T, BL], F8)          # X^T: xdt[k,p,t,b] = X[t,b,128k+p]
    xtb = inp("xtb", [3, 128, BL, D], F8)          # X t-grouped: [t0:128|128:256|256:365]
    st0 = inp("st0", [4, 128, BL])             # [h0;c0]^T k-tiled
    idxr = inp("idxr", [128, 3, BL], F32)      # indices replicated over partitions
    # replicated weights
    iota = inp("iota", [128, 1], F32)
    emb_s = inp("emb_s", [54, 16])
    emb_i = inp("emb_i", [128, 32, 64])
    emb_f = inp("emb_f", [33, 16])
    wc1p = inp("wc1p", [7, 128, 512])
    bc1t = inp("bc1t", [128, 4], F32)
    wc2 = inp("wc2", [4, 128, 384])
    bc2t = inp("bc2t", [128, 3], F32)
    wc3 = inp("wc3", [3, 128, 512])
    bc3t = inp("bc3t", [128, 4], F32)
    wa1a = inp("wa1a", [2, 128, 128], F8)
    nwa1b = inp("nwa1b", [4, 128, 128])        # -Wa1[256:768] k-tiled
    nba1c = inp("nba1c", [128, 1], F32)        # -ba1 as a column
    wa2 = inp("wa2", [128, 1])
    wihg = inp("wihg", [2, 128, 1024])         # Wih^T k-tiled, gate-scaled
    whhg = inp("whhg", [2, 128, 1024])         # Whh^T k-tiled, gate-scaled
    bgt2 = inp("bgt2", [128, 8], F32)          # gate-scaled bias, m-tiled
    wt1 = inp("wt1", [2, 128, 128])
    bt1t = inp("bt1t", [128, 1], F32)
    wt2 = inp("wt2", [128, 1])
    bt2r = inp("bt2r", [128, 1], F32)
    id16 = inp("id16", [128, 128])             # bf16 identity
    out = nc.dram_tensor("out", [NF, BL], F32, kind="ExternalOutput")

    with tile.TileContext(nc) as tc:
        with (
            tc.tile_pool(name="const", bufs=1) as cpool,
            tc.tile_pool(name="big", bufs=1) as bigpool,
            tc.tile_pool(name="state", bufs=1) as state,
            tc.tile_pool(name="ps", bufs=2, space="PSUM") as ps,
            tc.tile_pool(name="psnq", bufs=1, space="PSUM") as psnq,
            tc.tile_pool(name="psg", bufs=1, space="PSUM") as psg,
            tc.tile_pool(name="pssc", bufs=2, space="PSUM") as pssc,
            tc.tile_pool(name="pswt", bufs=1, space="PSUM") as pswt,
        ):
            # ---- persistent constants (issued from Act HWDGE; SP is busy
            # streaming xdt and big xtb transfers block their issuing seq) ----
            def ld(ap_dram, shape, dt=BF, tag=None):
                t_ = cpool.tile(shape, dt, tag=tag or ap_dram.name)
                nc.scalar.dma_start(t_[:], ap_dram)
                return t_

            def ldk(dram, nt, m, pool, dt=BF):
                t_ = pool.tile([128, nt, m], dt, tag=dram.name)
                nc.scalar.dma_start(
                    t_[:, :, :], dram[:, :, :].rearrange("n p m -> p n m"))
                return t_

            ones1 = cpool.tile([1, BL], BF, tag="ones1")
            nc.vector.memset(ones1[:], 1.0)

            # ---- persistent big tensors ----
            xtb_s = bigpool.tile([128, 3, BL, D], F8, tag="xtb")
            P_s = bigpool.tile([128, T, BL], BF, tag="P")

            # ---- persistent state ----
            outs17 = state.tile([128, 2, NF + 1, BL], BF, tag="outs")
            c32 = state.tile([128, 2, BL], F32, tag="c32")
            cb = state.tile([128, 2, BL], BF, tag="cb")
            e_sb0 = state.tile([40, 368], BF, tag="esb0")
            nc.vector.memset(e_sb0[:, :], 0.0)
            e_sb1 = state.tile([40, 368], BF, tag="esb1")
            nc.vector.memset(e_sb1[:, :], 0.0)
            zz0 = state.tile([40, 1], F32, tag="zz0")
            nc.vector.memset(zz0[:, :], 1.0)
            zz1 = state.tile([40, 1], F32, tag="zz1")
            nc.vector.memset(zz1[:, :], 1.0)
            h1 = state.tile([128, 2, 512], BF, tag="h1")

            # ================= prologue =================
            with (
                tc.tile_pool(name="pro", bufs=1) as pro,
                tc.tile_pool(name="proh", bufs=2) as work,
                tc.tile_pool(name="xchunk", bufs=8) as xchunk,
            ):
                idx_s = pro.tile([128, 3, BL], F32, tag="idx")
                nc.scalar.dma_start(idx_s[:], idxr[:, :, :])
                iota_s = pro.tile([128, 1], F32, tag="iota")
                nc.scalar.dma_start(iota_s[:], iota[:, :])
                embi_s = pro.tile([128, 32, 64], BF, tag="embi")
                nc.scalar.dma_start(embi_s[:, :, :], emb_i[:, :, :])
                wa1a_s = ldk(wa1a, 2, 128, pro, dt=F8)
                st0_s = ldk(st0, 4, BL, pro)
                embs_s = pro.tile([54, 16], BF, tag="embs")
                nc.scalar.dma_start(embs_s[:], emb_s[:, :])
                embf_s = pro.tile([33, 16], BF, tag="embf")
                nc.scalar.dma_start(embf_s[:], emb_f[:, :])
                wc1p_s = ldk(wc1p, 7, 512, pro)
                bc1_s = pro.tile([128, 4], F32, tag="bc1")
                nc.scalar.dma_start(bc1_s[:], bc1t[:, :])
                wc2_s = ldk(wc2, 4, 384, pro)
                bc2_s = pro.tile([128, 3], F32, tag="bc2")
                nc.scalar.dma_start(bc2_s[:], bc2t[:, :])
                wc3_s = ldk(wc3, 3, 512, pro)
                bc3_s = pro.tile([128, 4], F32, tag="bc3")
                nc.scalar.dma_start(bc3_s[:], bc3t[:, :])
                # embeddings -> xcond tiles 0..2; thought -> 3..6
                xcond = pro.tile([128, 7, BL], BF, tag="xcond")
                nc.vector.memset(xcond[:, 0:3, :], 0.0)
                for k in range(4):
                    nc.vector.tensor_copy(xcond[:, 3 + k, :], st0_s[:, k, :])

                def onehot_embed(col, table_s, nt, width, out_slice):
                    pe_out = ps.tile([width, BL], F32, tag="sm")
                    for k in range(nt):
                        oh = work.tile([128, BL], BF, tag="oh")
                        nc.vector.scalar_tensor_tensor(
                            oh[:], idx_s[:, col, :], float(-128 * k),
                            iota_s[:, :].broadcast_to((128, BL)),
                            op0=ALU.add, op1=ALU.is_equal)
                        kk = table_s.shape[0] if nt == 1 else 128
                        lhs = table_s[:, k, :] if nt > 1 else table_s[:, :]
                        nc.tensor.matmul(pe_out[:], lhs[:kk] if nt == 1 else lhs,
                                         oh[:kk] if nt == 1 else oh[:],
                                         start=(k == 0), stop=(k == nt - 1))
                    nc.vector.tensor_copy(out_slice, pe_out[:])

                onehot_embed(0, embs_s, 1, 16, xcond[0:16, 0, :])
                onehot_embed(1, embi_s, 32, 64, xcond[0:64, 1, :])
                onehot_embed(2, embf_s, 1, 16, xcond[0:16, 2, :])

                # conditioning MLP
                def mlp_layer(w_s, nk, x_s, mt, bias_s, relu, out_t):
                    for m in range(mt):
                        pe_o = ps.tile([128, BL], F32, tag="sm")
                        for k in range(nk):
                            nc.tensor.matmul(
                                pe_o[:], w_s[:, k, m * 128:(m + 1) * 128],
                                x_s[:, k, :],
                                start=(k == 0), stop=(k == nk - 1))
                        if relu:
                            nc.vector.tensor_scalar(
                                out=out_t[:, m, :], in0=pe_o[:],
                                scalar1=bias_s[:, m:m + 1], scalar2=0.0,
                                op0=ALU.add, op1=ALU.max)
                        else:
                            nc.vector.tensor_scalar_add(
                                out_t[:, m, :], pe_o[:],
                                bias_s[:, m:m + 1])

                ct1 = pro.tile([128, 4, BL], BF, tag="ct1")
                mlp_layer(wc1p_s, 7, xcond, 4, bc1_s, True, ct1)
                ct2 = pro.tile([128, 3, BL], BF, tag="ct2")
                mlp_layer(wc2_s, 4, ct1, 3, bc2_s, True, ct2)
                ct3f = pro.tile([128, 4, BL], F32, tag="ct3f")
                mlp_layer(wc3_s, 3, ct2, 4, bc3_s, False, ct3f)

                # initial state: h0 -> outs17 slot 0, c0 -> c32 + cb
                nc.vector.tensor_copy(outs17[:, :, 0, :], ct3f[:, 0:2, :])
                nc.vector.tensor_copy(cb[:, :, :], ct3f[:, 2:4, :])
                nc.vector.tensor_copy(c32[:, :, :], ct3f[:, 2:4, :])

                # P precompute: P[f, t, b] = sum_d X[t,b,d] Wa1a[d,f]
                # xdt streamed in big staged pieces (few DMAs; SP issues),
                # matmul'd in 8-t psum sub-chunks.
                PT = 46
                CH = 8
                cc = 0
                t0 = 0
                while t0 < T:
                    pn = min(PT, T - t0)
                    xc = xchunk.tile([128, 2, PT, BL], F8, tag="xc")
                    for k in range(2):
                        nc.sync.dma_start(xc[:, k, :pn, :],
                                          xdt[k, :, t0:t0 + pn, :])
                    u0 = 0
                    while u0 < pn:
                        tn = min(CH, pn - u0)
                        pe_p = ps.tile([128, CH * BL], F32, tag="sm")
                        for k in range(2):
                            nc.tensor.matmul(
                                pe_p[:, :tn * BL], wa1a_s[:, k, :],
                                xc[:, k, u0:u0 + tn, :].rearrange(
                                    "p t b -> p (t b)"),
                                start=(k == 0), stop=(k == 1))
                        dst = P_s[:, t0 + u0:t0 + u0 + tn, :].rearrange(
                            "p t b -> p (t b)")
                        if t0 + u0 < 180:
                            nc.scalar.activation(dst, pe_p[:, :tn * BL],
                                                 AF.Copy)
                        else:
                            nc.vector.tensor_copy(dst, pe_p[:, :tn * BL])
                        cc += 1
                        u0 += tn
                    t0 += pn

                # SP queue after the xdt pieces: step-0 nq/score weights,
                # then xtb (H0 batch-half first), then gate/final weights.
                def lds(ap_dram, shape, dt=BF, tag=None):
                    t_ = cpool.tile(shape, dt, tag=tag or ap_dram.name)
                    nc.sync.dma_start(t_[:], ap_dram)
                    return t_

                def ldks(dram, nt, m):
                    t_ = cpool.tile([128, nt, m], BF, tag=dram.name)
                    nc.sync.dma_start(
                        t_[:, :, :], dram[:, :, :].rearrange("n p m -> p n m"))
                    return t_

                wa1b_s = ldks(nwa1b, 4, 128)
                nba1_s = lds(nba1c[:, :], [128, 1], F32)
                wa2_s = lds(wa2[:, :], [128, 1])
                id16_s = lds(id16[:, :], [128, 128])
                for k in range(3):
                    nc.sync.dma_start(xtb_s[:, k, 0:32, :],
                                      xtb[k, :, 0:32, :])
                wihg_s = ldks(wihg, 2, 1024)
                whhg_s = ldks(whhg, 2, 1024)
                bgt_s = lds(bgt2[:, :], [128, 8], F32)
                for k in range(3):
                    nc.sync.dma_start(xtb_s[:, k, 32:64, :],
                                      xtb[k, :, 32:64, :])
                wt1_s = ldks(wt1, 2, 128)
                bt1_s = lds(bt1t[:, :], [128, 1], F32)
                wt2_s = lds(wt2[:, :], [128, 1])
                bt2_s = lds(bt2r[:, :], [128, 1], F32)
                # gate bias replicated over b once; psum gets pre-written
                # with it each step instead of a bias-row matmul
                bgrep = cpool.tile([128, 8, 32], F32, tag="bgrep")
                nc.vector.tensor_copy(
                    bgrep[:], bgt_s[:, :].unsqueeze(2).broadcast_to(
                        (128, 8, 32)))

            # ================= decoder steps =================
            # Two anti-phase batch halves: while one half runs its DVE max
            # phase, the other half runs softmax/context/gates/LSTM.
            _dec_cm = tc.tile_pool(name="dec", bufs=1)
            work = _dec_cm.__enter__()
            HB = 32
            TC3 = [(0, 122), (122, 122), (244, 121)]
            groups = [(0, 128), (1, 128), (2, 109)]
            e_half = [e_sb0, e_sb1]
            zz_half = [zz0, zz1]
            m0 = work.tile([128, T, 32], BF, tag="m0")
            m1 = work.tile([128, T, 32], BF, tag="m1")
            m_half = [m0, m1]

            def hslice(h):
                return slice(h * HB, (h + 1) * HB)

            def chain(mm, prev):
                if prev is not None:
                    add_dep_helper(mm.ins, prev.ins, sync=False,
                                   reason="psum accum order")
                return mm

            def nq_mms(s, h):
                hsl = hslice(h)
                hs = [outs17[:, 0, s, hsl], outs17[:, 1, s, hsl],
                      cb[:, 0, hsl], cb[:, 1, hsl]]
                nq_ps = psnq.tile([128, HB], F32, tag=f"nqp{h}")
                nc.scalar.activation(
                    nq_ps[:], nba1_s[:, :].broadcast_to((128, HB)), AF.Copy)
                mms = [(wa1b_s[:, 2, :], hs[2]), (wa1b_s[:, 3, :], hs[3]),
                       (wa1b_s[:, 0, :], hs[0]), (wa1b_s[:, 1, :], hs[1])]
                prev = None
                for gi, (wm, xm) in enumerate(mms):
                    prev = chain(nc.tensor.matmul(
                        nq_ps[:], wm, xm, start=False, stop=(gi == 3)),
                        prev)
                return nq_ps

            def nq_copy(h, nq_ps):
                nq_sb = work.tile([128, HB], BF, tag=f"nq{h}")
                nc.vector.tensor_copy(nq_sb[:], nq_ps[:])
                return nq_sb

            NBD = 32

            def b_maxchunks(s, h, nq_sb):
                mh = m_half[h]
                for b in range(NBD, HB):
                    nc.scalar.activation(
                        mh[:, 0:T, b], P_s[:, 0:T, h * HB + b], AF.Relu,
                        bias=q_sb[h][:, b:b + 1])
                for t0, tn in TC3:
                    nc.vector.tensor_tensor(
                        mh[:, t0:t0 + tn, 0:NBD],
                        P_s[:, t0:t0 + tn, h * HB:h * HB + NBD],
                        nq_sb[:, 0:NBD].unsqueeze(1).broadcast_to(
                            (128, tn, NBD)),
                        op=ALU.max)

            def b_scores(s, h):
                mh = m_half[h]
                sc_ps = pssc.tile([40, T], F32, tag="sc")
                for t in range(T):
                    nc.tensor.matmul(sc_ps[0:NBD, t:t + 1],
                                     mh[:, t, 0:NBD], wa2_s[:, :],
                                     start=True, stop=True)
                if NBD < HB:
                    for t in range(T):
                        nc.tensor.matmul(sc_ps[32:40, t:t + 1],
                                         mh[:, t, NBD:HB], wa2_s[:, :],
                                         start=True, stop=True)
                return sc_ps

            def c_soft(s, h, sc_ps):
                esb = e_half[h]
                zz = zz_half[h]
                nc.scalar.activation(esb[0:NBD, 0:T], sc_ps[0:NBD, :],
                                     AF.Exp, accum_out=zz[0:NBD, :])
                if NBD < HB:
                    nc.scalar.activation(esb[32:40, 0:T], sc_ps[32:40, :],
                                         AF.Exp, accum_out=zz[32:40, :])
                zrec = work.tile([40, 1], F32, tag=f"zr{h}")
                nc.vector.reciprocal(zrec[:], zz[:])
                nc.vector.tensor_scalar_mul(esb[:, 0:T], esb[:, 0:T],
                                            zrec[:, 0:1])
                wt_ps = pswt.tile([128, 3, 40], BF, tag="wtp")
                nc.tensor.transpose(wt_ps[:, 0, :], esb[:, 0:128],
                                    id16_s[0:40, 0:40])
                nc.tensor.transpose(wt_ps[:, 1, :], esb[:, 128:256],
                                    id16_s[0:40, 0:40])
                nc.tensor.transpose(wt_ps[0:112, 2, :], esb[:, 256:368],
                                    id16_s[0:40, 0:40])
                w_th = work.tile([128, 3, 40], BF, tag=f"wt{h}")
                nc.scalar.activation(w_th[:, 0:2, :], wt_ps[:, 0:2, :],
                                     AF.Copy)
                nc.scalar.activation(w_th[0:112, 2, :], wt_ps[0:112, 2, :],
                                     AF.Copy)
                return w_th

            def d_ctx(s, h, w_th):
                v_ps = ps.tile([128, 2, HB], F32, tag="sm")
                prev = None
                for b in range(HB):
                    gb = h * HB + b
                    wcol = b if b < NBD else b + 8
                    for dm in range(2):
                        for gi, (g, kn) in enumerate(groups):
                            prev = chain(nc.tensor.matmul(
                                v_ps[:, dm, b:b + 1],
                                xtb_s[0:kn, g, gb, dm * 128:(dm + 1) * 128],
                                w_th[0:kn, g, wcol:wcol + 1],
                                start=(gi == 0), stop=(gi == 2)), prev)
                inpT = work.tile([128, 2, HB], BF, tag=f"inp{h}")
                nc.scalar.activation(inpT[:], v_ps[:], AF.Copy)
                return inpT

            def e_gates(s, h, inpT):
                hsl = hslice(h)
                hs0 = outs17[:, 0, s, hsl]
                hs1 = outs17[:, 1, s, hsl]
                g_ps = psg.tile([128, 8 * HB], F32, tag="g")
                nc.scalar.activation(
                    g_ps[:].rearrange("p (m b) -> p m b", m=8), bgrep[:],
                    AF.Copy)
                prev = None
                for m in range(8):
                    sl_ = g_ps[:, m * HB:(m + 1) * HB]
                    gmms = [(whhg_s[:, 0, m * 128:(m + 1) * 128], hs0),
                            (whhg_s[:, 1, m * 128:(m + 1) * 128], hs1),
                            (wihg_s[:, 0, m * 128:(m + 1) * 128],
                             inpT[:, 0, :]),
                            (wihg_s[:, 1, m * 128:(m + 1) * 128],
                             inpT[:, 1, :])]
                    for gi, (wm, xm) in enumerate(gmms):
                        prev = chain(nc.tensor.matmul(
                            sl_, wm, xm, start=False, stop=(gi == 3)),
                            prev)
                ey = work.tile([128, 8 * HB], F32, tag=f"ey{h}")
                nc.scalar.activation(ey[:], g_ps[:], AF.Exp)
                return ey

            def f1_lstm(s, h, ey):
                # sigma = 1/(1+ey) for i,f,o; tanh_g = 2r-1; c update
                hsl = hslice(h)
                nc.gpsimd.tensor_scalar_add(ey[:], ey[:], 1.0)
                nc.vector.reciprocal(ey[:], ey[:])
                gt = work.tile([128, 2, HB], F32, tag=f"gt{h}")
                nc.gpsimd.tensor_scalar(
                    out=gt[:],
                    in0=ey[:, 4 * HB:6 * HB].rearrange("p (k b) -> p k b",
                                                       k=2),
                    scalar1=2.0, scalar2=-1.0, op0=ALU.mult, op1=ALU.add)
                t1 = work.tile([128, 2, HB], F32, tag=f"t1{h}")
                nc.vector.tensor_tensor(
                    t1[:],
                    ey[:, 2 * HB:4 * HB].rearrange("p (k b) -> p k b", k=2),
                    c32[:, :, hsl], op=ALU.mult)
                t2 = work.tile([128, 2, HB], F32, tag=f"t2{h}")
                nc.vector.tensor_tensor(
                    t2[:],
                    ey[:, 0:2 * HB].rearrange("p (k b) -> p k b", k=2),
                    gt[:], op=ALU.mult)
                nc.vector.tensor_tensor(c32[:, :, hsl], t1[:], t2[:],
                                        op=ALU.add)
                nc.gpsimd.tensor_copy(cb[:, :, hsl], c32[:, :, hsl])

            def f2_lstm(s, h, ey):
                # h = sigma_o * tanh(c) with tanh via exp(-2c)
                hsl = hslice(h)
                ec = work.tile([128, 2, HB], F32, tag=f"ec{h}")
                nc.scalar.activation(ec[:], c32[:, :, hsl], AF.Exp,
                                     scale=-2.0)
                nc.vector.tensor_scalar_add(ec[:], ec[:], 1.0)
                nc.vector.reciprocal(ec[:], ec[:])
                nc.vector.tensor_scalar(out=ec[:], in0=ec[:], scalar1=2.0,
                                        scalar2=-1.0, op0=ALU.mult,
                                        op1=ALU.add)
                nc.vector.tensor_tensor(
                    outs17[:, :, s + 1, hsl],
                    ey[:, 6 * HB:8 * HB].rearrange("p (k b) -> p k b", k=2),
                    ec[:], op=ALU.mult)

            q_sb = [None, None]
            nqp0 = nq_mms(0, 0)
            nq0sb = nq_copy(0, nqp0)
            nqp1 = nq_mms(0, 1)
            nq1sb = nq_copy(1, nqp1)
            for s in range(NF):
                b_maxchunks(s, 0, nq0sb)
                if s > 0:
                    nq1sb = nq_copy(1, nqp1)
                b_maxchunks(s, 1, nq1sb)
                sc0 = b_scores(s, 0)
                wt0 = c_soft(s, 0, sc0)
                inp0 = d_ctx(s, 0, wt0)
                ey0 = e_gates(s, 0, inp0)
                sc1 = b_scores(s, 1)
                wt1h = c_soft(s, 1, sc1)
                inp1 = d_ctx(s, 1, wt1h)
                ey1 = e_gates(s, 1, inp1)
                f1_lstm(s, 0, ey0)
                f2_lstm(s, 0, ey0)
                if s + 1 < NF:
                    nqp0 = nq_mms(s + 1, 0)
                f1_lstm(s, 1, ey1)
                f2_lstm(s, 1, ey1)
                if s + 1 < NF:
                    nq0sb = nq_copy(0, nqp0)
                    nqp1 = nq_mms(s + 1, 1)

            # ---- final MLP: td = relu(outs@Wt1+bt1)@Wt2 + bt2 ----
            for half in range(2):
                f_ps = psg.tile([128, 512], F32, tag="g")
                for k in range(2):
                    nc.tensor.matmul(
                        f_ps[:],
                        wt1_s[:, k, :],
                        outs17[:, k, 1 + half * 8:1 + (half + 1) * 8,
                               :].rearrange("p s b -> p (s b)"),
                        start=(k == 0), stop=(k == 1))
                nc.scalar.activation(h1[:, half, :], f_ps[:], AF.Relu,
                                     bias=bt1_s[:, :])
            td_ps = ps.tile([128, 8], F32, tag="sm")
            h1f = h1[:, :, :].rearrange("p h x -> p (h x)")
            for j in range(8):
                nc.tensor.matmul(td_ps[:, j:j + 1],
                                 h1f[:, 128 * j:128 * (j + 1)],
                                 wt2_s[:, :], start=True, stop=True)
            td_sb = work.tile([128, 8], F32, tag="tdsb")
            nc.vector.tensor_scalar_add(td_sb[:], td_ps[:], bt2_s[:, 0:1])
            nc.sync.dma_start(
                out.rearrange("(j two) b -> (two b) j", two=2), td_sb[:, :])
            _dec_cm.__exit__(None, None, None)

    nc.compile()
    return nc


_NC = None


def _get_nc():
    global _NC
    if _NC is None:
        _NC = build_bass()
    return _NC


def kernel(x_cat_static, state_h, state_c, outputs_encoder,
           emb_store, emb_item, emb_family,
           Wc1, bc1, Wc2, bc2, Wc3, bc3,
           Wa1, ba1, Wa2, ba2,
           Wt1, bt1, Wt2, bt2,
           Wih, Whh, bih, bhh):
    nc = _get_nc()

    # Wc1 rows: [store 16][item 64][family 16][thought 512] -> pad each
    # embedding block to its own 128-row K-tile, thought gets 4 tiles.
    wc1p = np.zeros((7, 128, 512), ml_dtypes.bfloat16)
    wc1p[0, :16] = _bf(Wc1[0:16])
    wc1p[1, :64] = _bf(Wc1[16:80])
    wc1p[2, :16] = _bf(Wc1[80:96])
    for k in range(4):
        wc1p[3 + k] = _bf(Wc1[96 + k * 128:96 + (k + 1) * 128])

    emb_i_t = np.zeros((32, 128, 64), ml_dtypes.bfloat16)
    ei = _bf(emb_item)
    for k in range(32):
        lo = k * 128
        hi = min(4036, lo + 128)
        if lo < 4036:
            emb_i_t[k, :hi - lo] = ei[lo:hi]
    emb_i_t = np.ascontiguousarray(emb_i_t.transpose(1, 0, 2))

    # gate scaling: sigmoid gates (i,f,o) need exp(-x); tanh gate (g) exp(-2x)
    gsc = np.full(1024, -1.0, np.float32)
    gsc[512:768] = -2.0
    wih_g = (np.asarray(Wih, np.float32) * gsc[:, None]).T   # [256, 1024]
    whh_g = (np.asarray(Whh, np.float32) * gsc[:, None]).T
    bias_g = (_f32(bih) + _f32(bhh)) * gsc

    common = {
        "iota": np.arange(128, dtype=np.float32).reshape(128, 1),
        "emb_s": _bf(emb_store),
        "emb_i": emb_i_t,
        "emb_f": _bf(emb_family),
        "wc1p": wc1p,
        "bc1t": _f32(bc1).reshape(4, 128).T.copy(),
        "wc2": _ktile(_bf(Wc2), 512, 512),
        "bc2t": _f32(bc2).reshape(3, 128).T.copy(),
        "wc3": _ktile(_bf(Wc3), 384, 384),
        "bc3t": _f32(bc3).reshape(4, 128).T.copy(),
        "wa1a": _ktile(np.ascontiguousarray(Wa1[:256]).astype(
            ml_dtypes.float8_e4m3), 256, 256),
        "nwa1b": _ktile(_bf(-np.asarray(Wa1[256:], np.float32)), 512, 512),
        "nba1c": (-_f32(ba1)).reshape(128, 1),
        "wa2": _bf(Wa2),
        "wihg": _ktile(_bf(wih_g), 256, 256),
        "whhg": _ktile(_bf(whh_g), 256, 256),
        "bgt2": np.ascontiguousarray(
            _f32(bias_g).reshape(8, 128).T),
        "wt1": _ktile(_bf(Wt1), 256, 256),
        "bt1t": _f32(bt1).reshape(128, 1),
        "wt2": _bf(Wt2),
        "bt2r": np.full((128, 1), float(np.asarray(bt2).reshape(-1)[0]),
                        np.float32),
        "id16": np.eye(128, dtype=ml_dtypes.bfloat16),
    }

    oe = np.asarray(outputs_encoder)
    th = np.concatenate([np.asarray(state_h)[0], np.asarray(state_c)[0]],
                        axis=-1)  # [B, 512]
    xc = np.asarray(x_cat_static)

    in_maps = []
    for c in range(N_CORES):
        b0 = c * BL
        shb = np.ascontiguousarray(oe[:, b0:b0 + BL, :]).astype(
            ml_dtypes.float8_e4m3)                     # [T, BL, D]
        xtb_t = np.zeros((3, 128, BL, D), ml_dtypes.float8_e4m3)
        xtb_t[0] = shb[0:128]
        xtb_t[1] = shb[128:256]
        xtb_t[2, 0:109] = shb[256:365]
        xdt_t = np.ascontiguousarray(
            oe[:, b0:b0 + BL, :].transpose(2, 0, 1).reshape(
                2, 128, T, BL)).astype(ml_dtypes.float8_e4m3)
        st_t = np.ascontiguousarray(
            _bf(th[b0:b0 + BL]).T.reshape(4, 128, BL))
        idxr = np.broadcast_to(
            xc[b0:b0 + BL].T.astype(np.float32)[None, :, :],
            (128, 3, BL)).copy()
        m = dict(common)
        m.update({"xdt": xdt_t, "xtb": xtb_t, "st0": st_t, "idxr": idxr})
        in_maps.append(m)

    kw = {}
    if os.environ.get("KTRACE", "") == "1":
        kw = dict(trace=True, trace_cores=[0])
    res = run_bass_kernel_spmd(nc, in_maps, list(range(N_CORES)), **kw)
    if res.exec_time_ns is not None:
        print("HW exec time:", res.exec_time_ns, "ns  (mean",
              res.mean_exec_time_ns, ")", flush=True)
    outs = [res.results[c]["out"] for c in range(N_CORES)]
    return np.concatenate(outs, axis=1).astype(np.float32)


# revision 39
# speedup vs baseline: 1.1565x; 1.1565x over previous
import sys, os
sys.path.insert(0, '/opt/trn_rl_repo')
import numpy as np
import ml_dtypes
import concourse.bass as bass
import concourse.bacc as bacc
import concourse.mybir as mybir
import concourse.tile as tile
from concourse.tile import add_dep_helper
from concourse.bass_utils import run_bass_kernel_spmd

BF = mybir.dt.bfloat16
F32 = mybir.dt.float32
F8 = mybir.dt.float8e4
AF = mybir.ActivationFunctionType
ALU = mybir.AluOpType
AX = mybir.AxisListType

N_CORES = 8
B = 512
BL = B // N_CORES      # 64 batch rows per core
T = 365
D = 256
FA = 128               # attention MLP hidden
NF = 16                # forecast steps
NR = 256               # decoder hidden

# max-phase split: DVE handles t[0:TDVE) in 4 chunks, Pool t[TDVE:365)
TDVE = 264
DCH = TDVE // 4        # 66


def _bf(x):
    return np.ascontiguousarray(x).astype(ml_dtypes.bfloat16)


def _f32(x):
    return np.ascontiguousarray(x).astype(np.float32)


def _ktile(w, k_total, pad_to):
    """[K, M] -> [ntiles, 128, M] zero-padded on K."""
    K, M = w.shape
    assert K == k_total
    nt = (pad_to + 127) // 128
    out = np.zeros((nt, 128, M), w.dtype)
    for i in range(nt):
        lo = i * 128
        hi = min(K, lo + 128)
        if lo < K:
            out[i, :hi - lo] = w[lo:hi]
    return out


def build_bass():
    nc = bacc.Bacc("TRN2", target_bir_lowering=False, debug=False,
                   num_devices=N_CORES)

    def inp(name, shape, dt=BF):
        return nc.dram_tensor(name, shape, dt, kind="ExternalInput")

    # per-core sharded tensors
    xdt = inp("xdt", [2, 128, T, BL], F8)          # X^T: xdt[k,p,t,b] = X[t,b,128k+p]
    xtb = inp("xtb", [3, 128, BL, D], F8)          # X t-grouped: [t0:128|128:256|256:365]
    st0 = inp("st0", [4, 128, BL])             # [h0;c0]^T k-tiled
    idxr = inp("idxr", [128, 3, BL], F32)      # indices replicated over partitions
    # replicated weights
    iota = inp("iota", [128, 1], F32)
    emb_s = inp("emb_s", [54, 16])
    emb_i = inp("emb_i", [128, 32, 64])
    emb_f = inp("emb_f", [33, 16])
    wc1p = inp("wc1p", [7, 128, 512])
    bc1t = inp("bc1t", [128, 4], F32)
    wc2 = inp("wc2", [4, 128, 384])
    bc2t = inp("bc2t", [128, 3], F32)
    wc3 = inp("wc3", [3, 128, 512])
    bc3t = inp("bc3t", [128, 4], F32)
    wa1a = inp("wa1a", [2, 128, 128], F8)
    nwa1b = inp("nwa1b", [4, 128, 128])        # -Wa1[256:768] k-tiled
    nba1c = inp("nba1c", [128, 1], F32)        # -ba1 as a column
    wa2 = inp("wa2", [128, 1])
    wihg = inp("wihg", [2, 128, 1024])         # Wih^T k-tiled, gate-scaled
    whhg = inp("whhg", [2, 128, 1024])         # Whh^T k-tiled, gate-scaled
    bgt2 = inp("bgt2", [128, 8], F32)          # gate-scaled bias, m-tiled
    wt1 = inp("wt1", [2, 128, 128])
    bt1t = inp("bt1t", [128, 1], F32)
    wt2 = inp("wt2", [128, 1])
    bt2r = inp("bt2r", [128, 1], F32)
    id16 = inp("id16", [128, 128])             # bf16 identity
    out = nc.dram_tensor("out", [NF, BL], F32, kind="ExternalOutput")

    with tile.TileContext(nc) as tc:
        with (
            tc.tile_pool(name="const", bufs=1) as cpool,
            tc.tile_pool(name="big", bufs=1) as bigpool,
            tc.tile_pool(name="state", bufs=1) as state,
            tc.tile_pool(name="ps", bufs=2, space="PSUM") as ps,
            tc.tile_pool(name="pps", bufs=2, space="PSUM") as pps,
            tc.tile_pool(name="psg", bufs=1, space="PSUM") as psg,
            tc.tile_pool(name="pssc", bufs=1, space="PSUM") as pssc,
            tc.tile_pool(name="pswt", bufs=1, space="PSUM") as pswt,
        ):
            # ---- persistent constants (issued from Act HWDGE; SP is busy
            # streaming xdt and big xtb transfers block their issuing seq) ----
            def ld(ap_dram, shape, dt=BF, tag=None):
                t_ = cpool.tile(shape, dt, tag=tag or ap_dram.name)
                nc.scalar.dma_start(t_[:], ap_dram)
                return t_

            def ldk(dram, nt, m, pool, dt=BF):
                t_ = pool.tile([128, nt, m], dt, tag=dram.name)
                nc.scalar.dma_start(
                    t_[:, :, :], dram[:, :, :].rearrange("n p m -> p n m"))
                return t_

            ones1 = cpool.tile([1, BL], BF, tag="ones1")
            nc.vector.memset(ones1[:], 1.0)

            # ---- persistent big tensors ----
            xtb_s = bigpool.tile([128, 3, BL, D], F8, tag="xtb")
            P_s = bigpool.tile([128, T, BL], BF, tag="P")

            # ---- persistent state ----
            outs17 = state.tile([128, 2, NF + 1, BL], BF, tag="outs")
            c32 = state.tile([128, 2, BL], F32, tag="c32")
            cb = state.tile([128, 2, BL], BF, tag="cb")
            e_sb0 = state.tile([32, 368], BF, tag="esb0")
            nc.vector.memset(e_sb0[:, T:368], 0.0)
            e_sb1 = state.tile([32, 368], BF, tag="esb1")
            nc.vector.memset(e_sb1[:, T:368], 0.0)
            h1 = state.tile([128, 2, 512], BF, tag="h1")

            # ================= prologue =================
            with (
                tc.tile_pool(name="pro", bufs=1) as pro,
                tc.tile_pool(name="proh", bufs=2) as work,
                tc.tile_pool(name="xchunk", bufs=8) as xchunk,
            ):
                idx_s = pro.tile([128, 3, BL], F32, tag="idx")
                nc.scalar.dma_start(idx_s[:], idxr[:, :, :])
                iota_s = pro.tile([128, 1], F32, tag="iota")
                nc.scalar.dma_start(iota_s[:], iota[:, :])
                embi_s = pro.tile([128, 32, 64], BF, tag="embi")
                nc.scalar.dma_start(embi_s[:, :, :], emb_i[:, :, :])
                wa1a_s = ldk(wa1a, 2, 128, pro, dt=F8)
                st0_s = ldk(st0, 4, BL, pro)
                embs_s = pro.tile([54, 16], BF, tag="embs")
                nc.scalar.dma_start(embs_s[:], emb_s[:, :])
                embf_s = pro.tile([33, 16], BF, tag="embf")
                nc.scalar.dma_start(embf_s[:], emb_f[:, :])
                wc1p_s = ldk(wc1p, 7, 512, pro)
                bc1_s = pro.tile([128, 4], F32, tag="bc1")
                nc.scalar.dma_start(bc1_s[:], bc1t[:, :])
                wc2_s = ldk(wc2, 4, 384, pro)
                bc2_s = pro.tile([128, 3], F32, tag="bc2")
                nc.scalar.dma_start(bc2_s[:], bc2t[:, :])
                wc3_s = ldk(wc3, 3, 512, pro)
                bc3_s = pro.tile([128, 4], F32, tag="bc3")
                nc.scalar.dma_start(bc3_s[:], bc3t[:, :])
                # embeddings -> xcond tiles 0..2; thought -> 3..6
                xcond = pro.tile([128, 7, BL], BF, tag="xcond")
                nc.vector.memset(xcond[:, 0:3, :], 0.0)
                for k in range(4):
                    nc.vector.tensor_copy(xcond[:, 3 + k, :], st0_s[:, k, :])

                def onehot_embed(col, table_s, nt, width, out_slice):
                    pe_out = ps.tile([width, BL], F32, tag="sm")
                    for k in range(nt):
                        oh = work.tile([128, BL], BF, tag="oh")
                        nc.vector.scalar_tensor_tensor(
                            oh[:], idx_s[:, col, :], float(-128 * k),
                            iota_s[:, :].broadcast_to((128, BL)),
                            op0=ALU.add, op1=ALU.is_equal)
                        kk = table_s.shape[0] if nt == 1 else 128
                        lhs = table_s[:, k, :] if nt > 1 else table_s[:, :]
                        nc.tensor.matmul(pe_out[:], lhs[:kk] if nt == 1 else lhs,
                                         oh[:kk] if nt == 1 else oh[:],
                                         start=(k == 0), stop=(k == nt - 1))
                    nc.vector.tensor_copy(out_slice, pe_out[:])

                onehot_embed(0, embs_s, 1, 16, xcond[0:16, 0, :])
                onehot_embed(1, embi_s, 32, 64, xcond[0:64, 1, :])
                onehot_embed(2, embf_s, 1, 16, xcond[0:16, 2, :])

                # conditioning MLP
                def mlp_layer(w_s, nk, x_s, mt, bias_s, relu, out_t):
                    for m in range(mt):
                        pe_o = ps.tile([128, BL], F32, tag="sm")
                        for k in range(nk):
                            nc.tensor.matmul(
                                pe_o[:], w_s[:, k, m * 128:(m + 1) * 128],
                                x_s[:, k, :],
                                start=(k == 0), stop=(k == nk - 1))
                        if relu:
                            nc.vector.tensor_scalar(
                                out=out_t[:, m, :], in0=pe_o[:],
                                scalar1=bias_s[:, m:m + 1], scalar2=0.0,
                                op0=ALU.add, op1=ALU.max)
                        else:
                            nc.vector.tensor_scalar_add(
                                out_t[:, m, :], pe_o[:],
                                bias_s[:, m:m + 1])

                ct1 = pro.tile([128, 4, BL], BF, tag="ct1")
                mlp_layer(wc1p_s, 7, xcond, 4, bc1_s, True, ct1)
                ct2 = pro.tile([128, 3, BL], BF, tag="ct2")
                mlp_layer(wc2_s, 4, ct1, 3, bc2_s, True, ct2)
                ct3f = pro.tile([128, 4, BL], F32, tag="ct3f")
                mlp_layer(wc3_s, 3, ct2, 4, bc3_s, False, ct3f)

                # initial state: h0 -> outs17 slot 0, c0 -> c32 + cb
                nc.vector.tensor_copy(outs17[:, :, 0, :], ct3f[:, 0:2, :])
                nc.vector.tensor_copy(cb[:, :, :], ct3f[:, 2:4, :])
                nc.vector.tensor_copy(c32[:, :, :], ct3f[:, 2:4, :])

                # P precompute: P[f, t, b] = sum_d X[t,b,d] Wa1a[d,f]
                # xdt streamed in big staged pieces (few DMAs; SP issues),
                # matmul'd in 8-t psum sub-chunks.
                PT = 46
                CH = 8
                cc = 0
                t0 = 0
                while t0 < T:
                    pn = min(PT, T - t0)
                    xc = xchunk.tile([128, 2, PT, BL], F8, tag="xc")
                    for k in range(2):
                        nc.sync.dma_start(xc[:, k, :pn, :],
                                          xdt[k, :, t0:t0 + pn, :])
                    u0 = 0
                    while u0 < pn:
                        tn = min(CH, pn - u0)
                        pe_p = pps.tile([128, CH * BL], F32, tag="pch")
                        for k in range(2):
                            nc.tensor.matmul(
                                pe_p[:, :tn * BL], wa1a_s[:, k, :],
                                xc[:, k, u0:u0 + tn, :].rearrange(
                                    "p t b -> p (t b)"),
                                start=(k == 0), stop=(k == 1))
                        dst = P_s[:, t0 + u0:t0 + u0 + tn, :].rearrange(
                            "p t b -> p (t b)")
                        if t0 + u0 < 180:
                            nc.scalar.activation(dst, pe_p[:, :tn * BL],
                                                 AF.Copy)
                        else:
                            nc.vector.tensor_copy(dst, pe_p[:, :tn * BL])
                        cc += 1
                        u0 += tn
                    t0 += pn

                # SP queue after the xdt pieces: step-0 nq/score weights,
                # then xtb (H0 batch-half first), then gate/final weights.
                def lds(ap_dram, shape, dt=BF, tag=None):
                    t_ = cpool.tile(shape, dt, tag=tag or ap_dram.name)
                    nc.sync.dma_start(t_[:], ap_dram)
                    return t_

                def ldks(dram, nt, m):
                    t_ = cpool.tile([128, nt, m], BF, tag=dram.name)
                    nc.sync.dma_start(
                        t_[:, :, :], dram[:, :, :].rearrange("n p m -> p n m"))
                    return t_

                wa1b_s = ldks(nwa1b, 4, 128)
                nba1_s = lds(nba1c[:, :], [128, 1], F32)
                wa2_s = lds(wa2[:, :], [128, 1])
                id16_s = lds(id16[:, :], [128, 128])
                for k in range(3):
                    nc.sync.dma_start(xtb_s[:, k, 0:32, :],
                                      xtb[k, :, 0:32, :])
                wihg_s = ldks(wihg, 2, 1024)
                whhg_s = ldks(whhg, 2, 1024)
                bgt_s = lds(bgt2[:, :], [128, 8], F32)
                for k in range(3):
                    nc.sync.dma_start(xtb_s[:, k, 32:64, :],
                                      xtb[k, :, 32:64, :])
                wt1_s = ldks(wt1, 2, 128)
                bt1_s = lds(bt1t[:, :], [128, 1], F32)
                wt2_s = lds(wt2[:, :], [128, 1])
                bt2_s = lds(bt2r[:, :], [128, 1], F32)
                # gate bias replicated over b once; psum gets pre-written
                # with it each step instead of a bias-row matmul
                bgrep = cpool.tile([128, 8, 32], F32, tag="bgrep")
                nc.vector.tensor_copy(
                    bgrep[:], bgt_s[:, :].unsqueeze(2).broadcast_to(
                        (128, 8, 32)))

            # ================= decoder steps =================
            # Two anti-phase batch halves: while one half runs its DVE max
            # phase, the other half runs softmax/context/gates/LSTM.
            _dec_cm = tc.tile_pool(name="dec", bufs=1)
            work = _dec_cm.__enter__()
            HB = 32
            TC3 = [(0, 122), (122, 122), (244, 121)]
            groups = [(0, 128), (1, 128), (2, 109)]
            e_half = [e_sb0, e_sb1]
            m0 = work.tile([128, T, 32], BF, tag="m0")
            m1 = work.tile([128, T, 32], BF, tag="m1")
            m_half = [m0, m1]

            def hslice(h):
                return slice(h * HB, (h + 1) * HB)

            def chain(mm, prev):
                if prev is not None:
                    add_dep_helper(mm.ins, prev.ins, sync=False,
                                   reason="psum accum order")
                return mm

            def a_nq(s, h):
                hsl = hslice(h)
                hs = [outs17[:, 0, s, hsl], outs17[:, 1, s, hsl],
                      cb[:, 0, hsl], cb[:, 1, hsl]]
                nq_ps = ps.tile([128, HB], F32, tag="sm")
                nc.scalar.activation(
                    nq_ps[:], nba1_s[:, :].broadcast_to((128, HB)), AF.Copy)
                mms = [(wa1b_s[:, 2, :], hs[2]), (wa1b_s[:, 3, :], hs[3]),
                       (wa1b_s[:, 0, :], hs[0]), (wa1b_s[:, 1, :], hs[1])]
                prev = None
                for gi, (wm, xm) in enumerate(mms):
                    prev = chain(nc.tensor.matmul(
                        nq_ps[:], wm, xm, start=False, stop=(gi == 3)),
                        prev)
                nq_sb = work.tile([128, HB], BF, tag=f"nq{h}")
                nc.vector.tensor_copy(nq_sb[:], nq_ps[:])
                q_sb = work.tile([128, HB], BF, tag=f"q{h}")
                nc.scalar.activation(q_sb[:], nq_ps[:], AF.Copy, scale=-1.0)
                return nq_sb, q_sb

            NBD = 24

            def b_max(s, h, nq_sb, q_sb):
                hsl = hslice(h)
                mh = m_half[h]
                sc_ps = pssc.tile([HB, T], F32, tag="sc")
                for b in range(NBD, HB):
                    nc.scalar.activation(
                        mh[:, 0:T, b], P_s[:, 0:T, h * HB + b], AF.Relu,
                        bias=q_sb[:, b:b + 1])
                for t0, tn in TC3:
                    nc.vector.tensor_tensor(
                        mh[:, t0:t0 + tn, 0:NBD],
                        P_s[:, t0:t0 + tn, h * HB:h * HB + NBD],
                        nq_sb[:, 0:NBD].unsqueeze(1).broadcast_to(
                            (128, tn, NBD)),
                        op=ALU.max)
                    for jj in range(tn):
                        nc.tensor.matmul(sc_ps[:, t0 + jj:t0 + jj + 1],
                                         mh[:, t0 + jj, :], wa2_s[:, :],
                                         start=True, stop=True)
                return sc_ps

            def c_soft(s, h, sc_ps):
                esb = e_half[h]
                zz = work.tile([HB, 1], F32, tag=f"zz{h}")
                nc.scalar.activation(esb[:, 0:T], sc_ps[:, :], AF.Exp,
                                     accum_out=zz[:])
                zrec = work.tile([HB, 1], F32, tag=f"zr{h}")
                nc.vector.reciprocal(zrec[:], zz[:])
                nc.vector.tensor_scalar_mul(esb[:, 0:T], esb[:, 0:T],
                                            zrec[:, 0:1])
                wt_ps = pswt.tile([128, 3, HB], BF, tag="wtp")
                nc.tensor.transpose(wt_ps[:, 0, :], esb[:, 0:128],
                                    id16_s[0:HB, 0:HB])
                nc.tensor.transpose(wt_ps[:, 1, :], esb[:, 128:256],
                                    id16_s[0:HB, 0:HB])
                nc.tensor.transpose(wt_ps[0:112, 2, :], esb[:, 256:368],
                                    id16_s[0:HB, 0:HB])
                w_th = work.tile([128, 3, HB], BF, tag=f"wt{h}")
                nc.scalar.activation(w_th[:, 0:2, :], wt_ps[:, 0:2, :],
                                     AF.Copy)
                nc.scalar.activation(w_th[0:112, 2, :], wt_ps[0:112, 2, :],
                                     AF.Copy)
                return w_th

            def d_ctx(s, h, w_th):
                v_ps = ps.tile([128, 2, HB], F32, tag="sm")
                prev = None
                for b in range(HB):
                    gb = h * HB + b
                    for dm in range(2):
                        for gi, (g, kn) in enumerate(groups):
                            prev = chain(nc.tensor.matmul(
                                v_ps[:, dm, b:b + 1],
                                xtb_s[0:kn, g, gb, dm * 128:(dm + 1) * 128],
                                w_th[0:kn, g, b:b + 1],
                                start=(gi == 0), stop=(gi == 2)), prev)
                inpT = work.tile([128, 2, HB], BF, tag=f"inp{h}")
                nc.scalar.activation(inpT[:], v_ps[:], AF.Copy)
                return inpT

            def e_gates(s, h, inpT):
                hsl = hslice(h)
                hs0 = outs17[:, 0, s, hsl]
                hs1 = outs17[:, 1, s, hsl]
                g_ps = psg.tile([128, 8 * HB], F32, tag="g")
                nc.scalar.activation(
                    g_ps[:].rearrange("p (m b) -> p m b", m=8), bgrep[:],
                    AF.Copy)
                prev = None
                for m in range(8):
                    sl_ = g_ps[:, m * HB:(m + 1) * HB]
                    gmms = [(whhg_s[:, 0, m * 128:(m + 1) * 128], hs0),
                            (whhg_s[:, 1, m * 128:(m + 1) * 128], hs1),
                            (wihg_s[:, 0, m * 128:(m + 1) * 128],
                             inpT[:, 0, :]),
                            (wihg_s[:, 1, m * 128:(m + 1) * 128],
                             inpT[:, 1, :])]
                    for gi, (wm, xm) in enumerate(gmms):
                        prev = chain(nc.tensor.matmul(
                            sl_, wm, xm, start=False, stop=(gi == 3)),
                            prev)
                ey = work.tile([128, 8 * HB], F32, tag=f"ey{h}")
                nc.scalar.activation(ey[:], g_ps[:], AF.Exp)
                return ey

            def f1_lstm(s, h, ey):
                # sigma = 1/(1+ey) for i,f,o; tanh_g = 2r-1; c update
                hsl = hslice(h)
                nc.gpsimd.tensor_scalar_add(ey[:], ey[:], 1.0)
                nc.vector.reciprocal(ey[:], ey[:])
                gt = work.tile([128, 2, HB], F32, tag=f"gt{h}")
                nc.gpsimd.tensor_scalar(
                    out=gt[:],
                    in0=ey[:, 4 * HB:6 * HB].rearrange("p (k b) -> p k b",
                                                       k=2),
                    scalar1=2.0, scalar2=-1.0, op0=ALU.mult, op1=ALU.add)
                t1 = work.tile([128, 2, HB], F32, tag=f"t1{h}")
                nc.vector.tensor_tensor(
                    t1[:],
                    ey[:, 2 * HB:4 * HB].rearrange("p (k b) -> p k b", k=2),
                    c32[:, :, hsl], op=ALU.mult)
                t2 = work.tile([128, 2, HB], F32, tag=f"t2{h}")
                nc.vector.tensor_tensor(
                    t2[:],
                    ey[:, 0:2 * HB].rearrange("p (k b) -> p k b", k=2),
                    gt[:], op=ALU.mult)
                nc.vector.tensor_tensor(c32[:, :, hsl], t1[:], t2[:],
                                        op=ALU.add)
                nc.gpsimd.tensor_copy(cb[:, :, hsl], c32[:, :, hsl])

            def f2_lstm(s, h, ey):
                # h = sigma_o * tanh(c) with tanh via exp(-2c)
                hsl = hslice(h)
                ec = work.tile([128, 2, HB], F32, tag=f"ec{h}")
                nc.scalar.activation(ec[:], c32[:, :, hsl], AF.Exp,
                                     scale=-2.0)
                nc.vector.tensor_scalar_add(ec[:], ec[:], 1.0)
                nc.vector.reciprocal(ec[:], ec[:])
                nc.vector.tensor_scalar(out=ec[:], in0=ec[:], scalar1=2.0,
                                        scalar2=-1.0, op0=ALU.mult,
                                        op1=ALU.add)
                nc.vector.tensor_tensor(
                    outs17[:, :, s + 1, hsl],
                    ey[:, 6 * HB:8 * HB].rearrange("p (k b) -> p k b", k=2),
                    ec[:], op=ALU.mult)

            nq0 = a_nq(0, 0)
            ey_prev = [None, None]
            for s in range(NF):
                sc0 = b_max(s, 0, *nq0)
                if s > 0:
                    f1_lstm(s - 1, 1, ey_prev[1])
                wt0 = c_soft(s, 0, sc0)
                if s > 0:
                    f2_lstm(s - 1, 1, ey_prev[1])
                nq1 = a_nq(s, 1)
                inp0 = d_ctx(s, 0, wt0)
                ey0 = e_gates(s, 0, inp0)
                sc1 = b_max(s, 1, *nq1)
                f1_lstm(s, 0, ey0)
                wt1h = c_soft(s, 1, sc1)
                f2_lstm(s, 0, ey0)
                if s + 1 < NF:
                    nq0 = a_nq(s + 1, 0)
                inp1 = d_ctx(s, 1, wt1h)
                ey_prev[1] = e_gates(s, 1, inp1)
            f1_lstm(NF - 1, 1, ey_prev[1])
            f2_lstm(NF - 1, 1, ey_prev[1])

            # ---- final MLP: td = relu(outs@Wt1+bt1)@Wt2 + bt2 ----
            for half in range(2):
                f_ps = psg.tile([128, 512], F32, tag="g")
                for k in range(2):
                    nc.tensor.matmul(
                        f_ps[:],
                        wt1_s[:, k, :],
                        outs17[:, k, 1 + half * 8:1 + (half + 1) * 8,
                               :].rearrange("p s b -> p (s b)"),
                        start=(k == 0), stop=(k == 1))
                nc.scalar.activation(h1[:, half, :], f_ps[:], AF.Relu,
                                     bias=bt1_s[:, :])
            td_ps = ps.tile([128, 8], F32, tag="sm")
            h1f = h1[:, :, :].rearrange("p h x -> p (h x)")
            for j in range(8):
                nc.tensor.matmul(td_ps[:, j:j + 1],
                                 h1f[:, 128 * j:128 * (j + 1)],
                                 wt2_s[:, :], start=True, stop=True)
            td_sb = work.tile([128, 8], F32, tag="tdsb")
            nc.vector.tensor_scalar_add(td_sb[:], td_ps[:], bt2_s[:, 0:1])
            nc.sync.dma_start(
                out.rearrange("(j two) b -> (two b) j", two=2), td_sb[:, :])
            _dec_cm.__exit__(None, None, None)

    nc.compile()
    return nc


_NC = None


def _get_nc():
    global _NC
    if _NC is None:
        _NC = build_bass()
    return _NC


def kernel(x_cat_static, state_h, state_c, outputs_encoder,
           emb_store, emb_item, emb_family,
           Wc1, bc1, Wc2, bc2, Wc3, bc3,
           Wa1, ba1, Wa2, ba2,
           Wt1, bt1, Wt2, bt2,
           Wih, Whh, bih, bhh):
    nc = _get_nc()

    # Wc1 rows: [store 16][item 64][family 16][thought 512] -> pad each
    # embedding block to its own 128-row K-tile, thought gets 4 tiles.
    wc1p = np.zeros((7, 128, 512), ml_dtypes.bfloat16)
    wc1p[0, :16] = _bf(Wc1[0:16])
    wc1p[1, :64] = _bf(Wc1[16:80])
    wc1p[2, :16] = _bf(Wc1[80:96])
    for k in range(4):
        wc1p[3 + k] = _bf(Wc1[96 + k * 128:96 + (k + 1) * 128])

    emb_i_t = np.zeros((32, 128, 64), ml_dtypes.bfloat16)
    ei = _bf(emb_item)
    for k in range(32):
        lo = k * 128
        hi = min(4036, lo + 128)
        if lo < 4036:
            emb_i_t[k, :hi - lo] = ei[lo:hi]
    emb_i_t = np.ascontiguousarray(emb_i_t.transpose(1, 0, 2))

    # gate scaling: sigmoid gates (i,f,o) need exp(-x); tanh gate (g) exp(-2x)
    gsc = np.full(1024, -1.0, np.float32)
    gsc[512:768] = -2.0
    wih_g = (np.asarray(Wih, np.float32) * gsc[:, None]).T   # [256, 1024]
    whh_g = (np.asarray(Whh, np.float32) * gsc[:, None]).T
    bias_g = (_f32(bih) + _f32(bhh)) * gsc

    common = {
        "iota": np.arange(128, dtype=np.float32).reshape(128, 1),
        "emb_s": _bf(emb_store),
        "emb_i": emb_i_t,
        "emb_f": _bf(emb_family),
        "wc1p": wc1p,
        "bc1t": _f32(bc1).reshape(4, 128).T.copy(),
        "wc2": _ktile(_bf(Wc2), 512, 512),
        "bc2t": _f32(bc2).reshape(3, 128).T.copy(),
        "wc3": _ktile(_bf(Wc3), 384, 384),
        "bc3t": _f32(bc3).reshape(4, 128).T.copy(),
        "wa1a": _ktile(np.ascontiguousarray(Wa1[:256]).astype(
            ml_dtypes.float8_e4m3), 256, 256),
        "nwa1b": _ktile(_bf(-np.asarray(Wa1[256:], np.float32)), 512, 512),
        "nba1c": (-_f32(ba1)).reshape(128, 1),
        "wa2": _bf(Wa2),
        "wihg": _ktile(_bf(wih_g), 256, 256),
        "whhg": _ktile(_bf(whh_g), 256, 256),
        "bgt2": np.ascontiguousarray(
            _f32(bias_g).reshape(8, 128).T),
        "wt1": _ktile(_bf(Wt1), 256, 256),
        "bt1t": _f32(bt1).reshape(128, 1),
        "wt2": _bf(Wt2),
        "bt2r": np.full((128, 1), float(np.asarray(bt2).reshape(-1)[0]),
                        np.float32),
        "id16": np.eye(128, dtype=ml_dtypes.bfloat16),
    }

    oe = np.asarray(outputs_encoder)
    th = np.concatenate([np.asarray(state_h)[0], np.asarray(state_c)[0]],
                        axis=-1)  # [B, 512]
    xc = np.asarray(x_cat_static)

    in_maps = []
    for c in range(N_CORES):
        b0 = c * BL
        shb = np.ascontiguousarray(oe[:, b0:b0 + BL, :]).astype(
            ml_dtypes.float8_e4m3)                     # [T, BL, D]
        xtb_t = np.zeros((3, 128, BL, D), ml_dtypes.float8_e4m3)
        xtb_t[0] = shb[0:128]
        xtb_t[1] = shb[128:256]
        xtb_t[2, 0:109] = shb[256:365]
        xdt_t = np.ascontiguousarray(
            oe[:, b0:b0 + BL, :].transpose(2, 0, 1).reshape(
                2, 128, T, BL)).astype(ml_dtypes.float8_e4m3)
        st_t = np.ascontiguousarray(
            _bf(th[b0:b0 + BL]).T.reshape(4, 128, BL))
        idxr = np.broadcast_to(
            xc[b0:b0 + BL].T.astype(np.float32)[None, :, :],
            (128, 3, BL)).copy()
        m = dict(common)
        m.update({"xdt": xdt_t, "xtb": xtb_t, "st0": st_t, "idxr": idxr})
        in_maps.append(m)

    kw = {}
    if os.environ.get("KTRACE", "") == "1":
        kw = dict(trace=True, trace_cores=[0])
    res = run_bass_kernel_spmd(nc, in_maps, list(range(N_CORES)), **kw)
    if res.exec_time_ns is not None:
        print("HW exec time:", res.exec_time_ns, "ns  (mean",
              res.mean_exec_time_ns, ")", flush=True)
    outs = [res.results[c]["out"] for c in range(N_CORES)]
    return np.concatenate(outs, axis=1).astype(np.float32)


# revision 40
# speedup vs baseline: 1.1628x; 1.0055x over previous
import sys, os
sys.path.insert(0, '/opt/trn_rl_repo')
import numpy as np
import ml_dtypes
import concourse.bass as bass
import concourse.bacc as bacc
import concourse.mybir as mybir
import concourse.tile as tile
from concourse.tile import add_dep_helper
from concourse.bass_utils import run_bass_kernel_spmd

BF = mybir.dt.bfloat16
F32 = mybir.dt.float32
F8 = mybir.dt.float8e4
AF = mybir.ActivationFunctionType
ALU = mybir.AluOpType
AX = mybir.AxisListType

N_CORES = 8
B = 512
BL = B // N_CORES      # 64 batch rows per core
T = 365
D = 256
FA = 128               # attention MLP hidden
NF = 16                # forecast steps
NR = 256               # decoder hidden

# max-phase split: DVE handles t[0:TDVE) in 4 chunks, Pool t[TDVE:365)
TDVE = 264
DCH = TDVE // 4        # 66


def _bf(x):
    return np.ascontiguousarray(x).astype(ml_dtypes.bfloat16)


def _f32(x):
    return np.ascontiguousarray(x).astype(np.float32)


def _ktile(w, k_total, pad_to):
    """[K, M] -> [ntiles, 128, M] zero-padded on K."""
    K, M = w.shape
    assert K == k_total
    nt = (pad_to + 127) // 128
    out = np.zeros((nt, 128, M), w.dtype)
    for i in range(nt):
        lo = i * 128
        hi = min(K, lo + 128)
        if lo < K:
            out[i, :hi - lo] = w[lo:hi]
    return out


def build_bass():
    nc = bacc.Bacc("TRN2", target_bir_lowering=False, debug=False,
                   num_devices=N_CORES)

    def inp(name, shape, dt=BF):
        return nc.dram_tensor(name, shape, dt, kind="ExternalInput")

    # per-core sharded tensors
    xdt = inp("xdt", [2, 128, T, BL], F8)          # X^T: xdt[k,p,t,b] = X[t,b,128k+p]
    xtb = inp("xtb", [3, 128, BL, D], F8)          # X t-grouped: [t0:128|128:256|256:365]
    st0 = inp("st0", [4, 128, BL])             # [h0;c0]^T k-tiled
    idxr = inp("idxr", [128, 3, BL], F32)      # indices replicated over partitions
    # replicated weights
    iota = inp("iota", [128, 1], F32)
    emb_s = inp("emb_s", [54, 16])
    emb_i = inp("emb_i", [128, 32, 64])
    emb_f = inp("emb_f", [33, 16])
    wc1p = inp("wc1p", [7, 128, 512])
    bc1t = inp("bc1t", [128, 4], F32)
    wc2 = inp("wc2", [4, 128, 384])
    bc2t = inp("bc2t", [128, 3], F32)
    wc3 = inp("wc3", [3, 128, 512])
    bc3t = inp("bc3t", [128, 4], F32)
    wa1a = inp("wa1a", [2, 128, 128], F8)
    nwa1b = inp("nwa1b", [4, 128, 128])        # -Wa1[256:768] k-tiled
    nba1c = inp("nba1c", [128, 1], F32)        # -ba1 as a column
    wa2 = inp("wa2", [128, 1])
    wihg = inp("wihg", [2, 128, 1024])         # Wih^T k-tiled, gate-scaled
    whhg = inp("whhg", [2, 128, 1024])         # Whh^T k-tiled, gate-scaled
    bgt2 = inp("bgt2", [128, 8], F32)          # gate-scaled bias, m-tiled
    wt1 = inp("wt1", [2, 128, 128])
    bt1t = inp("bt1t", [128, 1], F32)
    wt2 = inp("wt2", [128, 1])
    bt2r = inp("bt2r", [128, 1], F32)
    id16 = inp("id16", [128, 128])             # bf16 identity
    out = nc.dram_tensor("out", [NF, BL], F32, kind="ExternalOutput")

    with tile.TileContext(nc) as tc:
        with (
            tc.tile_pool(name="const", bufs=1) as cpool,
            tc.tile_pool(name="big", bufs=1) as bigpool,
            tc.tile_pool(name="state", bufs=1) as state,
            tc.tile_pool(name="ps", bufs=2, space="PSUM") as ps,
            tc.tile_pool(name="pps", bufs=2, space="PSUM") as pps,
            tc.tile_pool(name="psg", bufs=1, space="PSUM") as psg,
            tc.tile_pool(name="pssc", bufs=1, space="PSUM") as pssc,
            tc.tile_pool(name="pswt", bufs=1, space="PSUM") as pswt,
        ):
            # ---- persistent constants (issued from Act HWDGE; SP is busy
            # streaming xdt and big xtb transfers block their issuing seq) ----
            def ld(ap_dram, shape, dt=BF, tag=None):
                t_ = cpool.tile(shape, dt, tag=tag or ap_dram.name)
                nc.scalar.dma_start(t_[:], ap_dram)
                return t_

            def ldk(dram, nt, m, pool, dt=BF):
                t_ = pool.tile([128, nt, m], dt, tag=dram.name)
                nc.scalar.dma_start(
                    t_[:, :, :], dram[:, :, :].rearrange("n p m -> p n m"))
                return t_

            ones1 = cpool.tile([1, BL], BF, tag="ones1")
            nc.vector.memset(ones1[:], 1.0)

            # ---- persistent big tensors ----
            xtb_s = bigpool.tile([128, 3, BL, D], F8, tag="xtb")
            P_s = bigpool.tile([128, T, BL], BF, tag="P")

            # ---- persistent state ----
            outs17 = state.tile([128, 2, NF + 1, BL], BF, tag="outs")
            c32 = state.tile([128, 2, BL], F32, tag="c32")
            cb = state.tile([128, 2, BL], BF, tag="cb")
            e_sb0 = state.tile([32, 368], BF, tag="esb0")
            nc.vector.memset(e_sb0[:, T:368], 0.0)
            e_sb1 = state.tile([32, 368], BF, tag="esb1")
            nc.vector.memset(e_sb1[:, T:368], 0.0)
            h1 = state.tile([128, 2, 512], BF, tag="h1")

            # ================= prologue =================
            with (
                tc.tile_pool(name="pro", bufs=1) as pro,
                tc.tile_pool(name="proh", bufs=2) as work,
                tc.tile_pool(name="xchunk", bufs=8) as xchunk,
            ):
                idx_s = pro.tile([128, 3, BL], F32, tag="idx")
                nc.scalar.dma_start(idx_s[:], idxr[:, :, :])
                iota_s = pro.tile([128, 1], F32, tag="iota")
                nc.scalar.dma_start(iota_s[:], iota[:, :])
                embi_s = pro.tile([128, 32, 64], BF, tag="embi")
                nc.scalar.dma_start(embi_s[:, :, :], emb_i[:, :, :])
                wa1a_s = ldk(wa1a, 2, 128, pro, dt=F8)
                st0_s = ldk(st0, 4, BL, pro)
                embs_s = pro.tile([54, 16], BF, tag="embs")
                nc.scalar.dma_start(embs_s[:], emb_s[:, :])
                embf_s = pro.tile([33, 16], BF, tag="embf")
                nc.scalar.dma_start(embf_s[:], emb_f[:, :])
                wc1p_s = ldk(wc1p, 7, 512, pro)
                bc1_s = pro.tile([128, 4], F32, tag="bc1")
                nc.scalar.dma_start(bc1_s[:], bc1t[:, :])
                wc2_s = ldk(wc2, 4, 384, pro)
                bc2_s = pro.tile([128, 3], F32, tag="bc2")
                nc.scalar.dma_start(bc2_s[:], bc2t[:, :])
                wc3_s = ldk(wc3, 3, 512, pro)
                bc3_s = pro.tile([128, 4], F32, tag="bc3")
                nc.scalar.dma_start(bc3_s[:], bc3t[:, :])
                # embeddings -> xcond tiles 0..2; thought -> 3..6
                xcond = pro.tile([128, 7, BL], BF, tag="xcond")
                nc.vector.memset(xcond[:, 0:3, :], 0.0)
                for k in range(4):
                    nc.vector.tensor_copy(xcond[:, 3 + k, :], st0_s[:, k, :])

                def onehot_embed(col, table_s, nt, width, out_slice):
                    pe_out = ps.tile([width, BL], F32, tag="sm")
                    for k in range(nt):
                        oh = work.tile([128, BL], BF, tag="oh")
                        nc.vector.scalar_tensor_tensor(
                            oh[:], idx_s[:, col, :], float(-128 * k),
                            iota_s[:, :].broadcast_to((128, BL)),
                            op0=ALU.add, op1=ALU.is_equal)
                        kk = table_s.shape[0] if nt == 1 else 128
                        lhs = table_s[:, k, :] if nt > 1 else table_s[:, :]
                        nc.tensor.matmul(pe_out[:], lhs[:kk] if nt == 1 else lhs,
                                         oh[:kk] if nt == 1 else oh[:],
                                         start=(k == 0), stop=(k == nt - 1))
                    nc.vector.tensor_copy(out_slice, pe_out[:])

                onehot_embed(0, embs_s, 1, 16, xcond[0:16, 0, :])
                onehot_embed(1, embi_s, 32, 64, xcond[0:64, 1, :])
                onehot_embed(2, embf_s, 1, 16, xcond[0:16, 2, :])

                # conditioning MLP
                def mlp_layer(w_s, nk, x_s, mt, bias_s, relu, out_t):
                    for m in range(mt):
                        pe_o = ps.tile([128, BL], F32, tag="sm")
                        for k in range(nk):
                            nc.tensor.matmul(
                                pe_o[:], w_s[:, k, m * 128:(m + 1) * 128],
                                x_s[:, k, :],
                                start=(k == 0), stop=(k == nk - 1))
                        if relu:
                            nc.vector.tensor_scalar(
                                out=out_t[:, m, :], in0=pe_o[:],
                                scalar1=bias_s[:, m:m + 1], scalar2=0.0,
                                op0=ALU.add, op1=ALU.max)
                        else:
                            nc.vector.tensor_scalar_add(
                                out_t[:, m, :], pe_o[:],
                                bias_s[:, m:m + 1])

                ct1 = pro.tile([128, 4, BL], BF, tag="ct1")
                mlp_layer(wc1p_s, 7, xcond, 4, bc1_s, True, ct1)
                ct2 = pro.tile([128, 3, BL], BF, tag="ct2")
                mlp_layer(wc2_s, 4, ct1, 3, bc2_s, True, ct2)
                ct3f = pro.tile([128, 4, BL], F32, tag="ct3f")
                mlp_layer(wc3_s, 3, ct2, 4, bc3_s, False, ct3f)

                # initial state: h0 -> outs17 slot 0, c0 -> c32 + cb
                nc.vector.tensor_copy(outs17[:, :, 0, :], ct3f[:, 0:2, :])
                nc.vector.tensor_copy(cb[:, :, :], ct3f[:, 2:4, :])
                nc.vector.tensor_copy(c32[:, :, :], ct3f[:, 2:4, :])

                # P precompute: P[f, t, b] = sum_d X[t,b,d] Wa1a[d,f]
                # xdt streamed in big staged pieces (few DMAs; SP issues),
                # matmul'd in 8-t psum sub-chunks.
                PT = 46
                CH = 8
                cc = 0
                t0 = 0
                while t0 < T:
                    pn = min(PT, T - t0)
                    xc = xchunk.tile([128, 2, PT, BL], F8, tag="xc")
                    for k in range(2):
                        nc.sync.dma_start(xc[:, k, :pn, :],
                                          xdt[k, :, t0:t0 + pn, :])
                    u0 = 0
                    while u0 < pn:
                        tn = min(CH, pn - u0)
                        pe_p = pps.tile([128, CH * BL], F32, tag="pch")
                        for k in range(2):
                            nc.tensor.matmul(
                                pe_p[:, :tn * BL], wa1a_s[:, k, :],
                                xc[:, k, u0:u0 + tn, :].rearrange(
                                    "p t b -> p (t b)"),
                                start=(k == 0), stop=(k == 1))
                        dst = P_s[:, t0 + u0:t0 + u0 + tn, :].rearrange(
                            "p t b -> p (t b)")
                        if t0 + u0 < 250:
                            nc.vector.tensor_copy(dst, pe_p[:, :tn * BL])
                        else:
                            nc.scalar.activation(dst, pe_p[:, :tn * BL],
                                                 AF.Copy)
                        cc += 1
                        u0 += tn
                    t0 += pn

                # SP queue after the xdt pieces: step-0 nq/score weights,
                # then xtb (H0 batch-half first), then gate/final weights.
                def lds(ap_dram, shape, dt=BF, tag=None):
                    t_ = cpool.tile(shape, dt, tag=tag or ap_dram.name)
                    nc.sync.dma_start(t_[:], ap_dram)
                    return t_

                def ldks(dram, nt, m):
                    t_ = cpool.tile([128, nt, m], BF, tag=dram.name)
                    nc.sync.dma_start(
                        t_[:, :, :], dram[:, :, :].rearrange("n p m -> p n m"))
                    return t_

                wa1b_s = ldks(nwa1b, 4, 128)
                nba1_s = lds(nba1c[:, :], [128, 1], F32)
                wa2_s = lds(wa2[:, :], [128, 1])
                id16_s = lds(id16[:, :], [128, 128])
                for k in range(3):
                    nc.sync.dma_start(xtb_s[:, k, 0:32, :],
                                      xtb[k, :, 0:32, :])
                wihg_s = ldks(wihg, 2, 1024)
                whhg_s = ldks(whhg, 2, 1024)
                bgt_s = lds(bgt2[:, :], [128, 8], F32)
                for k in range(3):
                    nc.sync.dma_start(xtb_s[:, k, 32:64, :],
                                      xtb[k, :, 32:64, :])
                wt1_s = ldks(wt1, 2, 128)
                bt1_s = lds(bt1t[:, :], [128, 1], F32)
                wt2_s = lds(wt2[:, :], [128, 1])
                bt2_s = lds(bt2r[:, :], [128, 1], F32)
                # gate bias replicated over b once; psum gets pre-written
                # with it each step instead of a bias-row matmul
                bgrep = cpool.tile([128, 8, 32], F32, tag="bgrep")
                nc.vector.tensor_copy(
                    bgrep[:], bgt_s[:, :].unsqueeze(2).broadcast_to(
                        (128, 8, 32)))

            # ================= decoder steps =================
            # Two anti-phase batch halves: while one half runs its DVE max
            # phase, the other half runs softmax/context/gates/LSTM.
            _dec_cm = tc.tile_pool(name="dec", bufs=1)
            work = _dec_cm.__enter__()
            HB = 32
            TC3 = [(0, 122), (122, 122), (244, 121)]
            groups = [(0, 128), (1, 128), (2, 109)]
            e_half = [e_sb0, e_sb1]
            m0 = work.tile([128, T, 32], BF, tag="m0")
            m1 = work.tile([128, T, 32], BF, tag="m1")
            m_half = [m0, m1]

            def hslice(h):
                return slice(h * HB, (h + 1) * HB)

            def chain(mm, prev):
                if prev is not None:
                    add_dep_helper(mm.ins, prev.ins, sync=False,
                                   reason="psum accum order")
                return mm

            def a_nq(s, h):
                hsl = hslice(h)
                hs = [outs17[:, 0, s, hsl], outs17[:, 1, s, hsl],
                      cb[:, 0, hsl], cb[:, 1, hsl]]
                nq_ps = ps.tile([128, HB], F32, tag="sm")
                nc.scalar.activation(
                    nq_ps[:], nba1_s[:, :].broadcast_to((128, HB)), AF.Copy)
                mms = [(wa1b_s[:, 2, :], hs[2]), (wa1b_s[:, 3, :], hs[3]),
                       (wa1b_s[:, 0, :], hs[0]), (wa1b_s[:, 1, :], hs[1])]
                prev = None
                for gi, (wm, xm) in enumerate(mms):
                    prev = chain(nc.tensor.matmul(
                        nq_ps[:], wm, xm, start=False, stop=(gi == 3)),
                        prev)
                nq_sb = work.tile([128, HB], BF, tag=f"nq{h}")
                nc.vector.tensor_copy(nq_sb[:], nq_ps[:])
                q_sb = work.tile([128, HB], BF, tag=f"q{h}")
                nc.scalar.activation(q_sb[:], nq_ps[:], AF.Copy, scale=-1.0)
                return nq_sb, q_sb

            NBD = 24

            def b_max(s, h, nq_sb, q_sb):
                hsl = hslice(h)
                mh = m_half[h]
                sc_ps = pssc.tile([HB, T], F32, tag="sc")
                for b in range(NBD, HB):
                    nc.scalar.activation(
                        mh[:, 0:T, b], P_s[:, 0:T, h * HB + b], AF.Relu,
                        bias=q_sb[:, b:b + 1])
                for t0, tn in TC3:
                    nc.vector.tensor_tensor(
                        mh[:, t0:t0 + tn, 0:NBD],
                        P_s[:, t0:t0 + tn, h * HB:h * HB + NBD],
                        nq_sb[:, 0:NBD].unsqueeze(1).broadcast_to(
                            (128, tn, NBD)),
                        op=ALU.max)
                    for jj in range(tn):
                        nc.tensor.matmul(sc_ps[:, t0 + jj:t0 + jj + 1],
                                         mh[:, t0 + jj, :], wa2_s[:, :],
                                         start=True, stop=True)
                return sc_ps

            def c_soft(s, h, sc_ps):
                esb = e_half[h]
                zz = work.tile([HB, 1], F32, tag=f"zz{h}")
                nc.scalar.activation(esb[:, 0:T], sc_ps[:, :], AF.Exp,
                                     accum_out=zz[:])
                zrec = work.tile([HB, 1], F32, tag=f"zr{h}")
                nc.vector.reciprocal(zrec[:], zz[:])
                nc.vector.tensor_scalar_mul(esb[:, 0:T], esb[:, 0:T],
                                            zrec[:, 0:1])
                wt_ps = pswt.tile([128, 3, HB], BF, tag="wtp")
                nc.tensor.transpose(wt_ps[:, 0, :], esb[:, 0:128],
                                    id16_s[0:HB, 0:HB])
                nc.tensor.transpose(wt_ps[:, 1, :], esb[:, 128:256],
                                    id16_s[0:HB, 0:HB])
                nc.tensor.transpose(wt_ps[0:112, 2, :], esb[:, 256:368],
                                    id16_s[0:HB, 0:HB])
                w_th = work.tile([128, 3, HB], BF, tag=f"wt{h}")
                nc.scalar.activation(w_th[:, 0:2, :], wt_ps[:, 0:2, :],
                                     AF.Copy)
                nc.scalar.activation(w_th[0:112, 2, :], wt_ps[0:112, 2, :],
                                     AF.Copy)
                return w_th

            def d_ctx(s, h, w_th):
                v_ps = ps.tile([128, 2, HB], F32, tag="sm")
                prev = None
                for b in range(HB):
                    gb = h * HB + b
                    for dm in range(2):
                        for gi, (g, kn) in enumerate(groups):
                            prev = chain(nc.tensor.matmul(
                                v_ps[:, dm, b:b + 1],
                                xtb_s[0:kn, g, gb, dm * 128:(dm + 1) * 128],
                                w_th[0:kn, g, b:b + 1],
                                start=(gi == 0), stop=(gi == 2)), prev)
                inpT = work.tile([128, 2, HB], BF, tag=f"inp{h}")
                nc.scalar.activation(inpT[:], v_ps[:], AF.Copy)
                return inpT

            def e_gates(s, h, inpT):
                hsl = hslice(h)
                hs0 = outs17[:, 0, s, hsl]
                hs1 = outs17[:, 1, s, hsl]
                g_ps = psg.tile([128, 8 * HB], F32, tag="g")
                nc.scalar.activation(
                    g_ps[:].rearrange("p (m b) -> p m b", m=8), bgrep[:],
                    AF.Copy)
                prev = None
                for m in range(8):
                    sl_ = g_ps[:, m * HB:(m + 1) * HB]
                    gmms = [(whhg_s[:, 0, m * 128:(m + 1) * 128], hs0),
                            (whhg_s[:, 1, m * 128:(m + 1) * 128], hs1),
                            (wihg_s[:, 0, m * 128:(m + 1) * 128],
                             inpT[:, 0, :]),
                            (wihg_s[:, 1, m * 128:(m + 1) * 128],
                             inpT[:, 1, :])]
                    for gi, (wm, xm) in enumerate(gmms):
                        prev = chain(nc.tensor.matmul(
                            sl_, wm, xm, start=False, stop=(gi == 3)),
                            prev)
                ey = work.tile([128, 8 * HB], F32, tag=f"ey{h}")
                nc.scalar.activation(ey[:], g_ps[:], AF.Exp)
                return ey

            def f1_lstm(s, h, ey):
                # sigma = 1/(1+ey) for i,f,o; tanh_g = 2r-1; c update
                hsl = hslice(h)
                nc.gpsimd.tensor_scalar_add(ey[:], ey[:], 1.0)
                nc.vector.reciprocal(ey[:], ey[:])
                gt = work.tile([128, 2, HB], F32, tag=f"gt{h}")
                nc.gpsimd.tensor_scalar(
                    out=gt[:],
                    in0=ey[:, 4 * HB:6 * HB].rearrange("p (k b) -> p k b",
                                                       k=2),
                    scalar1=2.0, scalar2=-1.0, op0=ALU.mult, op1=ALU.add)
                t1 = work.tile([128, 2, HB], F32, tag=f"t1{h}")
                nc.vector.tensor_tensor(
                    t1[:],
                    ey[:, 2 * HB:4 * HB].rearrange("p (k b) -> p k b", k=2),
                    c32[:, :, hsl], op=ALU.mult)
                t2 = work.tile([128, 2, HB], F32, tag=f"t2{h}")
                nc.vector.tensor_tensor(
                    t2[:],
                    ey[:, 0:2 * HB].rearrange("p (k b) -> p k b", k=2),
                    gt[:], op=ALU.mult)
                nc.vector.tensor_tensor(c32[:, :, hsl], t1[:], t2[:],
                                        op=ALU.add)
                nc.gpsimd.tensor_copy(cb[:, :, hsl], c32[:, :, hsl])

            def f2_lstm(s, h, ey):
                # h = sigma_o * tanh(c) with tanh via exp(-2c)
                hsl = hslice(h)
                ec = work.tile([128, 2, HB], F32, tag=f"ec{h}")
                nc.scalar.activation(ec[:], c32[:, :, hsl], AF.Exp,
                                     scale=-2.0)
                nc.vector.tensor_scalar_add(ec[:], ec[:], 1.0)
                nc.vector.reciprocal(ec[:], ec[:])
                nc.vector.tensor_scalar(out=ec[:], in0=ec[:], scalar1=2.0,
                                        scalar2=-1.0, op0=ALU.mult,
                                        op1=ALU.add)
                nc.vector.tensor_tensor(
                    outs17[:, :, s + 1, hsl],
                    ey[:, 6 * HB:8 * HB].rearrange("p (k b) -> p k b", k=2),
                    ec[:], op=ALU.mult)

            nq0 = a_nq(0, 0)
            ey_prev = [None, None]
            for s in range(NF):
                sc0 = b_max(s, 0, *nq0)
                if s > 0:
                    f1_lstm(s - 1, 1, ey_prev[1])
                wt0 = c_soft(s, 0, sc0)
                if s > 0:
                    f2_lstm(s - 1, 1, ey_prev[1])
                nq1 = a_nq(s, 1)
                inp0 = d_ctx(s, 0, wt0)
                ey0 = e_gates(s, 0, inp0)
                sc1 = b_max(s, 1, *nq1)
                f1_lstm(s, 0, ey0)
                wt1h = c_soft(s, 1, sc1)
                f2_lstm(s, 0, ey0)
                if s + 1 < NF:
                    nq0 = a_nq(s + 1, 0)
                inp1 = d_ctx(s, 1, wt1h)
                ey_prev[1] = e_gates(s, 1, inp1)
            f1_lstm(NF - 1, 1, ey_prev[1])
            f2_lstm(NF - 1, 1, ey_prev[1])

            # ---- final MLP: td = relu(outs@Wt1+bt1)@Wt2 + bt2 ----
            for half in range(2):
                f_ps = psg.tile([128, 512], F32, tag="g")
                for k in range(2):
                    nc.tensor.matmul(
                        f_ps[:],
                        wt1_s[:, k, :],
                        outs17[:, k, 1 + half * 8:1 + (half + 1) * 8,
                               :].rearrange("p s b -> p (s b)"),
                        start=(k == 0), stop=(k == 1))
                nc.scalar.activation(h1[:, half, :], f_ps[:], AF.Relu,
                                     bias=bt1_s[:, :])
            td_ps = ps.tile([128, 8], F32, tag="sm")
            h1f = h1[:, :, :].rearrange("p h x -> p (h x)")
            for j in range(8):
                nc.tensor.matmul(td_ps[:, j:j + 1],
                                 h1f[:, 128 * j:128 * (j + 1)],
                                 wt2_s[:, :], start=True, stop=True)
            td_sb = work.tile([128, 8], F32, tag="tdsb")
            nc.vector.tensor_scalar_add(td_sb[:], td_ps[:], bt2_s[:, 0:1])
            nc.sync.dma_start(
                out.rearrange("(j two) b -> (two b) j", two=2), td_sb[:, :])
            _dec_cm.__exit__(None, None, None)

    nc.compile()
    return nc


_NC = None


def _get_nc():
    global _NC
    if _NC is None:
        _NC = build_bass()
    return _NC


def kernel(x_cat_static, state_h, state_c, outputs_encoder,
           emb_store, emb_item, emb_family,
           Wc1, bc1, Wc2, bc2, Wc3, bc3,
           Wa1, ba1, Wa2, ba2,
           Wt1, bt1, Wt2, bt2,
           Wih, Whh, bih, bhh):
    nc = _get_nc()

    # Wc1 rows: [store 16][item 64][family 16][thought 512] -> pad each
    # embedding block to its own 128-row K-tile, thought gets 4 tiles.
    wc1p = np.zeros((7, 128, 512), ml_dtypes.bfloat16)
    wc1p[0, :16] = _bf(Wc1[0:16])
    wc1p[1, :64] = _bf(Wc1[16:80])
    wc1p[2, :16] = _bf(Wc1[80:96])
    for k in range(4):
        wc1p[3 + k] = _bf(Wc1[96 + k * 128:96 + (k + 1) * 128])

    emb_i_t = np.zeros((32, 128, 64), ml_dtypes.bfloat16)
    ei = _bf(emb_item)
    for k in range(32):
        lo = k * 128
        hi = min(4036, lo + 128)
        if lo < 4036:
            emb_i_t[k, :hi - lo] = ei[lo:hi]
    emb_i_t = np.ascontiguousarray(emb_i_t.transpose(1, 0, 2))

    # gate scaling: sigmoid gates (i,f,o) need exp(-x); tanh gate (g) exp(-2x)
    gsc = np.full(1024, -1.0, np.float32)
    gsc[512:768] = -2.0
    wih_g = (np.asarray(Wih, np.float32) * gsc[:, None]).T   # [256, 1024]
    whh_g = (np.asarray(Whh, np.float32) * gsc[:, None]).T
    bias_g = (_f32(bih) + _f32(bhh)) * gsc

    common = {
        "iota": np.arange(128, dtype=np.float32).reshape(128, 1),
        "emb_s": _bf(emb_store),
        "emb_i": emb_i_t,
        "emb_f": _bf(emb_family),
        "wc1p": wc1p,
        "bc1t": _f32(bc1).reshape(4, 128).T.copy(),
        "wc2": _ktile(_bf(Wc2), 512, 512),
        "bc2t": _f32(bc2).reshape(3, 128).T.copy(),
        "wc3": _ktile(_bf(Wc3), 384, 384),
        "bc3t": _f32(bc3).reshape(4, 128).T.copy(),
        "wa1a": _ktile(np.ascontiguousarray(Wa1[:256]).astype(
            ml_dtypes.float8_e4m3), 256, 256),
        "nwa1b": _ktile(_bf(-np.asarray(Wa1[256:], np.float32)), 512, 512),
        "nba1c": (-_f32(ba1)).reshape(128, 1),
        "wa2": _bf(Wa2),
        "wihg": _ktile(_bf(wih_g), 256, 256),
        "whhg": _ktile(_bf(whh_g), 256, 256),
        "bgt2": np.ascontiguousarray(
            _f32(bias_g).reshape(8, 128).T),
        "wt1": _ktile(_bf(Wt1), 256, 256),
        "bt1t": _f32(bt1).reshape(128, 1),
        "wt2": _bf(Wt2),
        "bt2r": np.full((128, 1), float(np.asarray(bt2).reshape(-1)[0]),
                        np.float32),
        "id16": np.eye(128, dtype=ml_dtypes.bfloat16),
    }

    oe = np.asarray(outputs_encoder)
    th = np.concatenate([np.asarray(state_h)[0], np.asarray(state_c)[0]],
                        axis=-1)  # [B, 512]
    xc = np.asarray(x_cat_static)

    in_maps = []
    for c in range(N_CORES):
        b0 = c * BL
        shb = np.ascontiguousarray(oe[:, b0:b0 + BL, :]).astype(
            ml_dtypes.float8_e4m3)                     # [T, BL, D]
        xtb_t = np.zeros((3, 128, BL, D), ml_dtypes.float8_e4m3)
        xtb_t[0] = shb[0:128]
        xtb_t[1] = shb[128:256]
        xtb_t[2, 0:109] = shb[256:365]
        xdt_t = np.ascontiguousarray(
            oe[:, b0:b0 + BL, :].transpose(2, 0, 1).reshape(
                2, 128, T, BL)).astype(ml_dtypes.float8_e4m3)
        st_t = np.ascontiguousarray(
            _bf(th[b0:b0 + BL]).T.reshape(4, 128, BL))
        idxr = np.broadcast_to(
            xc[b0:b0 + BL].T.astype(np.float32)[None, :, :],
            (128, 3, BL)).copy()
        m = dict(common)
        m.update({"xdt": xdt_t, "xtb": xtb_t, "st0": st_t, "idxr": idxr})
        in_maps.append(m)

    kw = {}
    if os.environ.get("KTRACE", "") == "1":
        kw = dict(trace=True, trace_cores=[0])
    res = run_bass_kernel_spmd(nc, in_maps, list(range(N_CORES)), **kw)
    if res.exec_time_ns is not None:
        print("HW exec time:", res.exec_time_ns, "ns  (mean",
              res.mean_exec_time_ns, ")", flush=True)
    outs = [res.results[c]["out"] for c in range(N_CORES)]
    return np.concatenate(outs, axis=1).astype(np.float32)
